# revision 55
# baseline (speedup 1.0000x reference)
"""Multi-head self-attention (causal) Trainium2 Bass/Tile kernel, 8-way SPMD.

Sharding: data-parallel over batch (4) x tensor-parallel over heads (2 groups
of 8 heads).  Core c handles batch c//2, head-group c%2.  Each core computes
q/k/v projections for its 512 local features, causal attention for its 8
heads, and a partial o-projection (contraction over its 512 features of the
attention output) giving a full-shape [S, D] partial that the host sums per
batch pair.

All matmul operands are bf16 (fp32 PSUM accumulation); softmax runs without
max-subtraction (scores ~ N(0,1) after the 1/8 scale, no overflow risk), with
exp on the scalar engine and the row-sum folded into the AV matmul via a ones
column appended to V.  Host pre-transposes inputs so no on-chip transposes
are needed:
  qT[e,s]  = wqT.T @ xT        (lhsT=wqT[d,e], rhs=xT[d,s])
  scoresT[sk,sq] = kT.T @ qT   (lhsT=kT[dk,sk], rhs=qT[dk,sq], K=64)
  avT[dk+1,sq]   = vaug.T @ expT  (lhsT=vaug[sk,65], rhs=expT[sk,sq])
  y[s,e]   = outT.T @ woT      (lhsT=outT[d,s], rhs=woT[d,e])
Causal masking multiplies exp tiles of the 4 diagonal strips by precomputed
0/1 masks; off-diagonal upper tiles are skipped entirely.
"""

from contextlib import ExitStack

import numpy as np
import ml_dtypes

import concourse.bass as bass
import concourse.tile as tile
from concourse import bacc, mybir
from concourse._compat import with_exitstack
from concourse.bass_utils import run_bass_kernel_spmd

B, S, D, H = 4, 2048, 1024, 16
DK = D // H          # 64
E = 512              # local features per core (8 heads)
HL = 8               # local heads
NCORES = 8
NDT = D // 128       # 8 d-tiles
NET = E // 128       # 4 e-tiles
NST = S // 128       # 16 s-tiles
NQG = S // 512       # 4 query groups

F32 = mybir.dt.float32
BF16 = mybir.dt.bfloat16
bf16 = ml_dtypes.bfloat16

_compiled = None
last_results = None  # test harness introspection


@with_exitstack
def _mhsa_kernel(ctx: ExitStack, tc: tile.TileContext, y, xT, wqT, wkT, wvT,
                 woT, masks2):
    nc = tc.nc

    consts = ctx.enter_context(tc.tile_pool(name="consts", bufs=1))
    ex_pool = ctx.enter_context(tc.tile_pool(name="ex", bufs=8))
    rec_pool = ctx.enter_context(tc.tile_pool(name="rec", bufs=2))
    y_pool = ctx.enter_context(tc.tile_pool(name="ysb", bufs=3))
    ps_pool = ctx.enter_context(tc.tile_pool(name="psmm", bufs=3, space="PSUM"))
    av_pool = ctx.enter_context(tc.tile_pool(name="psav", bufs=2, space="PSUM"))

    def ctile(shape, dt_, tg):
        return consts.tile(shape, dt_, tag=tg, name=tg)

    # ---- persistent SBUF tiles -------------------------------------------
    xT_t = [ctile([128, S], BF16, f"xT{i}") for i in range(NDT)]
    wqT_t = [ctile([128, E], BF16, f"wqT{i}") for i in range(NDT)]
    wkT_t = [ctile([128, E], BF16, f"wkT{i}") for i in range(NDT)]
    wvT_t = [ctile([128, E], BF16, f"wvT{i}") for i in range(NDT)]
    woT_t = [ctile([128, D], BF16, f"woT{i}") for i in range(NET)]
    qT_t = [ctile([128, S], BF16, f"qT{i}") for i in range(NET)]
    kT_t = [ctile([128, S], BF16, f"kT{i}") for i in range(NET)]
    vaug_t = [ctile([128, HL * (DK + 1)], BF16, f"vaug{i}") for i in range(NST)]
    outT_t = [ctile([128, S], BF16, f"outT{i}") for i in range(NET)]
    mask_t = [ctile([128, 1024], BF16, f"mask{i}") for i in range(4)]

    # ---- input loads, ordered so compute can start ASAP ------------------
    # v-proj of the first s-tiles needs wvT + first xT columns; q/k of et0
    # needs wqT/wkT + xT.  xT streams on the gpsimd queues in parallel with
    # the weights on the sync queues.
    for c in range(2):
        for i in range(NDT):
            nc.gpsimd.dma_start(
                out=xT_t[i][:, c * 1024:(c + 1) * 1024],
                in_=xT[i * 128:(i + 1) * 128, c * 1024:(c + 1) * 1024])
    for i in range(NDT):
        nc.sync.dma_start(out=wvT_t[i], in_=wvT[i * 128:(i + 1) * 128, :])
    for i in range(NDT):
        nc.sync.dma_start(out=wqT_t[i], in_=wqT[i * 128:(i + 1) * 128, :])
        nc.sync.dma_start(out=wkT_t[i], in_=wkT[i * 128:(i + 1) * 128, :])
    for p in range(4):
        nc.sync.dma_start(out=mask_t[p], in_=masks2[p])
    for i in range(NET):
        nc.sync.dma_start(out=woT_t[i], in_=woT[i * 128:(i + 1) * 128, :])

    # ---- q/k projections: qT[e,s], kT[e,s] -------------------------------
    def proj_qk(wt, dst, et, scg):
        ps = ps_pool.tile([128, 1024], F32, tag="mm", name="ps")
        for dt_ in range(NDT):
            for hf in range(2):
                s0 = scg * 1024 + hf * 512
                nc.tensor.matmul(
                    ps[:, hf * 512:(hf + 1) * 512],
                    lhsT=wt[dt_][:, et * 128:(et + 1) * 128],
                    rhs=xT_t[dt_][:, s0:s0 + 512],
                    start=(dt_ == 0), stop=(dt_ == NDT - 1),
                )
        nc.vector.tensor_copy(dst[et][:, scg * 1024:(scg + 1) * 1024], ps)

    # ---- v projection -> vaug tiles [128, 8*65] with ones columns --------
    def proj_v(stp):
        ps = ps_pool.tile([128, 1024], F32, tag="mm", name="ps")
        for dt_ in range(NDT):
            for hf in range(2):
                st = 2 * stp + hf
                nc.tensor.matmul(
                    ps[:, hf * 512:(hf + 1) * 512],
                    lhsT=xT_t[dt_][:, st * 128:(st + 1) * 128],
                    rhs=wvT_t[dt_],
                    start=(dt_ == 0), stop=(dt_ == NDT - 1),
                )
        for hf in range(2):
            st = 2 * stp + hf
            nc.vector.memset(vaug_t[st], 1.0)
            # one strided cast: [128, 8, 64] view skips the ones columns
            nc.vector.tensor_copy(
                vaug_t[st].rearrange("p (h c) -> p h c", c=65)[:, :, 0:64],
                ps[:, hf * 512:(hf + 1) * 512].rearrange(
                    "p (h c) -> p h c", c=64),
            )

    # Softmax denominators bounce through DRAM: DVE can only write at
    # 32-aligned base partitions, and SBUF APs cannot have a step-0
    # partition dim (needed for the broadcast) — DRAM APs can do both.
    sums_dram = nc.dram_tensor("sums_bounce", [NQG, HL, 512], F32).ap()
    rec_dram = nc.dram_tensor("rec_bounce", [NQG, HL, 512], BF16).ap()

    # ones2: selector for the final pair's reciprocal broadcast matmul —
    # bc[j, :] = recb2[0, :] for j<64 (head A) and recb2[32, :] for j>=64
    # (head B).  Rows 0/32 because the DVE can only write at 32-aligned
    # partitions; K padded to 64 (a K=33 matmul wedges the exec unit).
    ones2 = ctile([64, 128], BF16, "ones2")
    nc.vector.memset(ones2, 0.0)
    nc.vector.memset(ones2[0:1, 0:64], 1.0)
    nc.vector.memset(ones2[32:33, 64:128], 1.0)

    # PE warm-up: HAM starts throttled at 1.2 GHz and needs ~3.4us of
    # sustained matmul activity to release; burn idle DMA-wait time at
    # kernel start so the first real matmuls run at full clock.
    warm = ctile([128, 512], BF16, "warm")
    nc.vector.memset(warm, 0.0)
    for _ in range(24):
        wps = ps_pool.tile([128, 512], F32, tag="mm", name="wps")
        nc.tensor.matmul(wps, lhsT=warm[:, 0:128], rhs=warm,
                         start=True, stop=True)

    # ---- attention for one (head-pair, query-group) ----------------------
    # Heads hA=2*hp (partitions 0:64) and hB=2*hp+1 (64:128) share each
    # score tile: [:, 0:512]=A, [:, 512:1024]=B for one key tile kt.  The
    # K=64 score matmuls for A and B land on disjoint PE row groups (base
    # partition 0 vs 64) and run concurrently.  outT stays UNNORMALIZED;
    # denominators are collected and normalization is batched per qg so
    # the PE never waits on the reciprocal chain.
    def attn(hp, qg):
        ti = hp
        hA, hB = 2 * hp, 2 * hp + 1
        nk = 4 * qg + 4
        avA = av_pool.tile([65, 512], F32, tag="av", name="avA")
        avB = av_pool.tile([65, 512], F32, tag="av", name="avB")

        def emit_av(kt, ex):
            for av, h in ((avA, hA), (avB, hB)):
                nc.tensor.matmul(
                    av,
                    lhsT=vaug_t[kt][:, h * 65:h * 65 + 65],
                    rhs=ex[:, (h & 1) * 512:((h & 1) + 1) * 512],
                    start=(kt == 0), stop=(kt == nk - 1),
                )

        pending = []
        for kt in range(nk):
            ps = ps_pool.tile([128, 1024], F32, tag="mm", name="ps")
            for po in (0, 64):
                nc.tensor.matmul(
                    ps[:, (po // 64) * 512:(po // 64 + 1) * 512],
                    lhsT=kT_t[ti][po:po + 64, kt * 128:(kt + 1) * 128],
                    rhs=qT_t[ti][po:po + 64, qg * 512:(qg + 1) * 512],
                    start=True, stop=True,
                )
            ex = ex_pool.tile([128, 1024], BF16, tag="ex", name="ex")
            r = kt - 4 * qg
            if qg >= 1 and r >= 2:
                # steep diagonal tiles: exp only the valid column range of
                # each head's half (the mask zeroes the stale rest; safe
                # because qg0's full-width exps already initialized every
                # ex-pool slot, so stale data is finite)
                for h0 in (0, 512):
                    nc.scalar.activation(
                        out=ex[:, h0 + 128 * r:h0 + 512],
                        in_=ps[:, h0 + 128 * r:h0 + 512],
                        func=mybir.ActivationFunctionType.Exp, scale=0.125)
            else:
                nc.scalar.activation(out=ex, in_=ps,
                                     func=mybir.ActivationFunctionType.Exp,
                                     scale=0.125)
            if r >= 0:  # diagonal strip: causal 0/1 mask (dup halves)
                nc.vector.tensor_mul(ex, ex, mask_t[r])
            pending.append((kt, ex))
            if len(pending) > 2:  # lag 2: AV never waits on a fresh exp
                emit_av(*pending.pop(0))

        # The remaining AV matmuls wait on the freshest exps; returning them
        # as a closure lets the caller slip an independent filler group in
        # front, so the PE chews filler instead of stalling on the ACT.
        def flush_tail():
            for item in pending:
                emit_av(*item)
            _stash(hp, qg, ti, avA, avB)
        return flush_tail

    def _stash(hp, qg, ti, avA, avB):
        # stash unnormalized outputs + denominators; release av quickly
        hA, hB = 2 * hp, 2 * hp + 1
        if qg == NQG - 1 and hp == HL // 2 - 1:
            # final pair: no attention left to hide the DRAM-bounce latency
            # behind, so normalize inline via reciprocal + PE broadcast
            stg2 = rec_pool.tile([64, 512], F32, tag="stg2", name="stg2")
            nc.vector.memset(stg2, 1.0)
            for av, po, row in ((avA, 0, 0), (avB, 64, 32)):
                nc.vector.tensor_copy(
                    outT_t[ti][po:po + 64, qg * 512:(qg + 1) * 512],
                    av[0:64, :])
                nc.vector.tensor_copy(stg2[row:row + 1, :], av[64:65, :])
            rec2 = rec_pool.tile([64, 512], F32, tag="rec2", name="rec2")
            nc.vector.reciprocal_approx_fast(out=rec2, in_=stg2)
            recb2 = rec_pool.tile([64, 512], BF16, tag="recb2", name="recb2")
            nc.vector.tensor_copy(recb2, rec2)
            bc = av_pool.tile([128, 512], F32, tag="av", name="bc")
            nc.tensor.matmul(bc, lhsT=ones2, rhs=recb2, start=True, stop=True)
            for po in (0, 64):
                sl = outT_t[ti][po:po + 64, qg * 512:(qg + 1) * 512]
                nc.vector.tensor_mul(sl, sl, bc[po:po + 64, :])
        else:
            for av, h, po in ((avA, hA, 0), (avB, hB, 64)):
                nc.vector.tensor_copy(
                    outT_t[ti][po:po + 64, qg * 512:(qg + 1) * 512],
                    av[0:64, :])
                stg = rec_pool.tile([1, 512], F32, tag="stg", name="stg",
                                    bufs=4)
                nc.vector.tensor_copy(stg, av[64:65, :])
                nc.sync.dma_start(out=sums_dram[qg, h], in_=stg)

    # ---- batched normalization (DRAM-bounce broadcast) -------------------
    def _norm_heads(qg, heads):
        h0, nh = heads[0], len(heads)
        sums = rec_pool.tile([nh, 512], F32, tag=f"sums{nh}", name="sums")
        nc.sync.dma_start(out=sums, in_=sums_dram[qg, h0:h0 + nh])
        rec = rec_pool.tile([nh, 512], F32, tag=f"rec{nh}", name="rec")
        nc.vector.reciprocal_approx_fast(out=rec, in_=sums)
        recb = rec_pool.tile([nh, 512], BF16, tag=f"recb{nh}", name="recb")
        nc.vector.tensor_copy(recb, rec)
        nc.sync.dma_start(out=rec_dram[qg, h0:h0 + nh], in_=recb)
        for h in heads:
            ti, po = h // 2, 64 * (h % 2)
            # walrus requires SBUF tensor_tensor inputs to share the start
            # partition, so land the broadcast at the same partition range
            bcs = rec_pool.tile([128, 512], BF16, tag="bcs", name="bcs")
            nc.sync.dma_start(
                out=bcs[po:po + 64, :],
                in_=rec_dram[qg, h:h + 1, :].to_broadcast([64, 512]))
            sl = outT_t[ti][po:po + 64, qg * 512:(qg + 1) * 512]
            nc.vector.tensor_mul(sl, sl, bcs[po:po + 64, :])

    def normalize(qg):
        _norm_heads(qg, list(range(HL)))

    def normalize_pair(qg, hp):
        _norm_heads(qg, [2 * hp, 2 * hp + 1])

    # ---- o-projection: y[s,:] partial ------------------------------------
    # ---- o-projection: y[s,:] partial ------------------------------------
    def oproj(st):
        ps = ps_pool.tile([128, 1024], F32, tag="mm", name="ps")
        for dt_ in range(NET):
            for hf in range(2):
                nc.tensor.matmul(
                    ps[:, hf * 512:(hf + 1) * 512],
                    lhsT=outT_t[dt_][:, st * 128:(st + 1) * 128],
                    rhs=woT_t[dt_][:, hf * 512:(hf + 1) * 512],
                    start=(dt_ == 0), stop=(dt_ == NET - 1),
                )
        ysb = y_pool.tile([128, 1024], F32, tag="ysb", name="ysb")
        # split copy+DMA per half so the writeback starts earlier
        for hf in range(2):
            nc.vector.tensor_copy(ysb[:, hf * 512:(hf + 1) * 512],
                                  ps[:, hf * 512:(hf + 1) * 512])
            nc.gpsimd.dma_start(
                out=y[st * 128:(st + 1) * 128, hf * 512:(hf + 1) * 512],
                in_=ysb[:, hf * 512:(hf + 1) * 512])

    # ---- program order ----------------------------------------------------
    # Attention is ACT(exp)-bound, so start it as soon as its first
    # dependencies exist (qg0 needs only q/k et0 cols 0:512 and v st0..3)
    # and drip the remaining PE-only projection work as filler between
    # head-pair slots, where it soaks up the PE's wait-on-exp slack.
    def qk_pair(et, scg):
        proj_qk(wqT_t, qT_t, et, scg)
        proj_qk(wkT_t, kT_t, et, scg)

    proj_v(0)
    proj_v(1)
    qk_pair(0, 0)

    fillers = {
        (0, 0): [lambda: qk_pair(1, 0), lambda: proj_v(2)],
        (0, 1): [lambda: qk_pair(2, 0), lambda: proj_v(3)],
        (0, 2): [lambda: qk_pair(3, 0), lambda: proj_v(4)],
        (0, 3): [lambda: proj_v(5)],
        (1, 0): [lambda: proj_v(6)],
        (1, 1): [lambda: proj_v(7)],
        (1, 2): [lambda: qk_pair(0, 1)],
        (1, 3): [lambda: qk_pair(1, 1)],
        (2, 0): [lambda: qk_pair(2, 1), lambda: normalize(0)],
        (2, 1): [lambda: qk_pair(3, 1), lambda: oproj(0)],
        (2, 2): [lambda: normalize(1), lambda: oproj(1)],
        (2, 3): [lambda: oproj(2), lambda: oproj(3)],
        # NOTE: a slot's fillers run AFTER its stash (flush_tail) — the
        # sums_bounce RAW dep is ordered only by sync-queue program order,
        # so normalize_pair(qg, hp) must never precede its own stash.
        (3, 0): [lambda: normalize_pair(3, 0), lambda: normalize(2),
                 lambda: oproj(4)],
        (3, 1): [lambda: normalize_pair(3, 1), lambda: oproj(5),
                 lambda: oproj(6)],
        (3, 2): [lambda: normalize_pair(3, 2), lambda: oproj(7),
                 lambda: oproj(8)],
        (3, 3): [lambda: oproj(9), lambda: oproj(10), lambda: oproj(11)],
    }
    for qg in range(NQG):
        for hp in range(HL // 2):
            flush_tail = attn(hp, qg)
            flush_tail()
            for f in fillers.get((qg, hp), []):
                f()
    for st in range(4 * (NQG - 1), 4 * NQG):
        oproj(st)


def _build():
    nc = bacc.Bacc("TRN2", target_bir_lowering=False, debug=False,
                   num_devices=NCORES)
    xT = nc.dram_tensor("xT", [D, S], BF16, kind="ExternalInput").ap()
    wqT = nc.dram_tensor("wqT", [D, E], BF16, kind="ExternalInput").ap()
    wkT = nc.dram_tensor("wkT", [D, E], BF16, kind="ExternalInput").ap()
    wvT = nc.dram_tensor("wvT", [D, E], BF16, kind="ExternalInput").ap()
    woT = nc.dram_tensor("woT", [E, D], BF16, kind="ExternalInput").ap()
    masks2 = nc.dram_tensor("masks2", [4, 128, 1024], BF16,
                            kind="ExternalInput").ap()
    y = nc.dram_tensor("y", [S, D], F32, kind="ExternalOutput").ap()
    with tile.TileContext(nc) as tc:
        _mhsa_kernel(tc, y, xT, wqT, wkT, wvT, woT, masks2)
    nc.compile()
    return nc


def get_compiled():
    global _compiled
    if _compiled is None:
        _compiled = _build()
    return _compiled


def _make_masks():
    # masks2[r][i, :] keeps key 128*r+i <= query j within the 512-wide
    # query group; duplicated in both 512-halves (head A | head B).
    m = np.zeros((4, 128, 1024), dtype=np.float32)
    col = np.arange(512)
    for r in range(4):
        half = (col[None, :] >= (128 * r + np.arange(128))[:, None])
        m[r, :, 0:512] = half
        m[r, :, 512:1024] = half
    return m.astype(bf16)


def kernel(**inputs):
    global last_results
    x = np.asarray(inputs["in_features"], dtype=np.float32)
    w_q = np.asarray(inputs["w_q"], dtype=np.float32)
    w_k = np.asarray(inputs["w_k"], dtype=np.float32)
    w_v = np.asarray(inputs["w_v"], dtype=np.float32)
    w_o = np.asarray(inputs["w_o"], dtype=np.float32)

    nc = get_compiled()
    masks2 = _make_masks()
    in_maps = []
    for c in range(NCORES):
        b, hg = divmod(c, 2)
        es = slice(hg * E, (hg + 1) * E)
        in_maps.append({
            "xT": x[b].T.astype(bf16),
            "wqT": w_q[es, :].T.astype(bf16),
            "wkT": w_k[es, :].T.astype(bf16),
            "wvT": w_v[es, :].T.astype(bf16),
            "woT": w_o[:, es].T.astype(bf16),
            "masks2": masks2,
        })
    res = run_bass_kernel_spmd(nc, in_maps, list(range(NCORES)))
    last_results = res
    y = np.zeros((B, S, D), dtype=np.float32)
    for c in range(NCORES):
        y[c // 2] += res.results[c]["y"]
    return y


# revision 56
# speedup vs baseline: 1.1544x; 1.1544x over previous
"""Multi-head self-attention (causal) Trainium2 Bass/Tile kernel, 8-way SPMD.

Sharding: data-parallel over batch (4) x tensor-parallel over heads (2 groups
of 8 heads).  Core c handles batch c//2, head-group c%2.  Each core computes
q/k/v projections for its 512 local features, causal attention for its 8
heads, and a partial o-projection (contraction over its 512 features of the
attention output) giving a full-shape [S, D] partial that the host sums per
batch pair.

All matmul operands are bf16 (fp32 PSUM accumulation); softmax runs without
max-subtraction (scores ~ N(0,1) after the 1/8 scale, no overflow risk), with
exp on the scalar engine and the row-sum folded into the AV matmul via a ones
column appended to V.  Host pre-transposes inputs so no on-chip transposes
are needed:
  qT[e,s]  = wqT.T @ xT        (lhsT=wqT[d,e], rhs=xT[d,s])
  scoresT[sk,sq] = kT.T @ qT   (lhsT=kT[dk,sk], rhs=qT[dk,sq], K=64)
  avT[dk+1,sq]   = vaug.T @ expT  (lhsT=vaug[sk,65], rhs=expT[sk,sq])
  y[s,e]   = outT.T @ woT      (lhsT=outT[d,s], rhs=woT[d,e])
Causal masking multiplies exp tiles of the 4 diagonal strips by precomputed
0/1 masks; off-diagonal upper tiles are skipped entirely.
"""

from contextlib import ExitStack

import numpy as np
import ml_dtypes

import concourse.bass as bass
import concourse.tile as tile
from concourse import bacc, mybir
from concourse._compat import with_exitstack
from concourse.bass_utils import run_bass_kernel_spmd

B, S, D, H = 4, 2048, 1024, 16
DK = D // H          # 64
E = 512              # local features per core (8 heads)
HL = 8               # local heads
NCORES = 8
NDT = D // 128       # 8 d-tiles
NET = E // 128       # 4 e-tiles
NST = S // 128       # 16 s-tiles
NQG = S // 512       # 4 query groups

F32 = mybir.dt.float32
BF16 = mybir.dt.bfloat16
bf16 = ml_dtypes.bfloat16

_compiled = None
last_results = None  # test harness introspection


@with_exitstack
def _mhsa_kernel(ctx: ExitStack, tc: tile.TileContext, y, xT, wqT, wkT, wvT,
                 woT, masks2):
    nc = tc.nc

    consts = ctx.enter_context(tc.tile_pool(name="consts", bufs=1))
    ex_pool = ctx.enter_context(tc.tile_pool(name="ex", bufs=8))
    rec_pool = ctx.enter_context(tc.tile_pool(name="rec", bufs=2))
    y_pool = ctx.enter_context(tc.tile_pool(name="ysb", bufs=3))
    ps_pool = ctx.enter_context(tc.tile_pool(name="psmm", bufs=3, space="PSUM"))
    av_pool = ctx.enter_context(tc.tile_pool(name="psav", bufs=2, space="PSUM"))

    def ctile(shape, dt_, tg):
        return consts.tile(shape, dt_, tag=tg, name=tg)

    # ---- persistent SBUF tiles -------------------------------------------
    xT_t = [ctile([128, S], BF16, f"xT{i}") for i in range(NDT)]
    wqT_t = [ctile([128, E], BF16, f"wqT{i}") for i in range(NDT)]
    wkT_t = [ctile([128, E], BF16, f"wkT{i}") for i in range(NDT)]
    wvT_t = [ctile([128, E], BF16, f"wvT{i}") for i in range(NDT)]
    woT_t = [ctile([128, D], BF16, f"woT{i}") for i in range(NET)]
    qT_t = [ctile([128, S], BF16, f"qT{i}") for i in range(NET)]
    kT_t = [ctile([128, S], BF16, f"kT{i}") for i in range(NET)]
    vaug_t = [ctile([128, HL * (DK + 1)], BF16, f"vaug{i}") for i in range(NST)]
    outT_t = [ctile([128, S], BF16, f"outT{i}") for i in range(NET)]
    mask_t = [ctile([128, 1024], BF16, f"mask{i}") for i in range(4)]

    # ---- input loads, ordered so compute can start ASAP ------------------
    # v-proj of the first s-tiles needs wvT + first xT columns; q/k of et0
    # needs wqT/wkT + xT.  xT streams on the gpsimd queues in parallel with
    # the weights on the sync queues.
    for c in range(2):
        for i in range(NDT):
            nc.gpsimd.dma_start(
                out=xT_t[i][:, c * 1024:(c + 1) * 1024],
                in_=xT[i * 128:(i + 1) * 128, c * 1024:(c + 1) * 1024])
    for i in range(NDT):
        nc.sync.dma_start(out=wvT_t[i], in_=wvT[i * 128:(i + 1) * 128, :])
    for i in range(NDT):
        nc.sync.dma_start(out=wqT_t[i], in_=wqT[i * 128:(i + 1) * 128, :])
        nc.sync.dma_start(out=wkT_t[i], in_=wkT[i * 128:(i + 1) * 128, :])
    for p in range(4):
        nc.sync.dma_start(out=mask_t[p], in_=masks2[p])
    for i in range(NET):
        nc.sync.dma_start(out=woT_t[i], in_=woT[i * 128:(i + 1) * 128, :])

    # ---- q/k projections: qT[e,s], kT[e,s] -------------------------------
    def proj_qk(wt, dst, et, scg):
        ps = ps_pool.tile([128, 1024], F32, tag="mm", name="ps")
        for dt_ in range(NDT):
            for hf in range(2):
                s0 = scg * 1024 + hf * 512
                nc.tensor.matmul(
                    ps[:, hf * 512:(hf + 1) * 512],
                    lhsT=wt[dt_][:, et * 128:(et + 1) * 128],
                    rhs=xT_t[dt_][:, s0:s0 + 512],
                    start=(dt_ == 0), stop=(dt_ == NDT - 1),
                )
        nc.vector.tensor_copy(dst[et][:, scg * 1024:(scg + 1) * 1024], ps)

    # ---- v projection -> vaug tiles [128, 8*65] with ones columns --------
    def proj_v(stp):
        ps = ps_pool.tile([128, 1024], F32, tag="mm", name="ps")
        for dt_ in range(NDT):
            for hf in range(2):
                st = 2 * stp + hf
                nc.tensor.matmul(
                    ps[:, hf * 512:(hf + 1) * 512],
                    lhsT=xT_t[dt_][:, st * 128:(st + 1) * 128],
                    rhs=wvT_t[dt_],
                    start=(dt_ == 0), stop=(dt_ == NDT - 1),
                )
        for hf in range(2):
            st = 2 * stp + hf
            nc.vector.memset(vaug_t[st], 1.0)
            # one strided cast: [128, 8, 64] view skips the ones columns
            nc.vector.tensor_copy(
                vaug_t[st].rearrange("p (h c) -> p h c", c=65)[:, :, 0:64],
                ps[:, hf * 512:(hf + 1) * 512].rearrange(
                    "p (h c) -> p h c", c=64),
            )

    # Softmax denominators bounce through DRAM: DVE can only write at
    # 32-aligned base partitions, and SBUF APs cannot have a step-0
    # partition dim (needed for the broadcast) — DRAM APs can do both.
    sums_dram = nc.dram_tensor("sums_bounce", [NQG, HL, 512], F32).ap()
    rec_dram = nc.dram_tensor("rec_bounce", [NQG, HL, 512], BF16).ap()

    # ones2: selector for the final pair's reciprocal broadcast matmul —
    # bc[j, :] = recb2[0, :] for j<64 (head A) and recb2[32, :] for j>=64
    # (head B).  Rows 0/32 because the DVE can only write at 32-aligned
    # partitions; K padded to 64 (a K=33 matmul wedges the exec unit).
    ones2 = ctile([64, 128], BF16, "ones2")
    nc.vector.memset(ones2, 0.0)
    nc.vector.memset(ones2[0:1, 0:64], 1.0)
    nc.vector.memset(ones2[32:33, 64:128], 1.0)

    # PE warm-up: HAM starts throttled at 1.2 GHz and needs ~3.4us of
    # sustained matmul activity to release; burn idle DMA-wait time at
    # kernel start so the first real matmuls run at full clock.
    warm = ctile([128, 512], BF16, "warm")
    nc.vector.memset(warm, 0.0)
    for _ in range(24):
        wps = ps_pool.tile([128, 512], F32, tag="mm", name="wps")
        nc.tensor.matmul(wps, lhsT=warm[:, 0:128], rhs=warm,
                         start=True, stop=True)

    # ---- attention for one (head-pair, query-group) ----------------------
    # Heads hA=2*hp (partitions 0:64) and hB=2*hp+1 (64:128) share each
    # score tile: [:, 0:512]=A, [:, 512:1024]=B for one key tile kt.  The
    # K=64 score matmuls for A and B land on disjoint PE row groups (base
    # partition 0 vs 64) and run concurrently.  outT stays UNNORMALIZED;
    # denominators are collected and normalization is batched per qg so
    # the PE never waits on the reciprocal chain.
    def attn(hp, qg):
        ti = hp
        hA, hB = 2 * hp, 2 * hp + 1
        nk = 4 * qg + 4
        avA = av_pool.tile([65, 512], F32, tag="av", name="avA")
        avB = av_pool.tile([65, 512], F32, tag="av", name="avB")

        def emit_av(kt, ex):
            for av, h in ((avA, hA), (avB, hB)):
                nc.tensor.matmul(
                    av,
                    lhsT=vaug_t[kt][:, h * 65:h * 65 + 65],
                    rhs=ex[:, (h & 1) * 512:((h & 1) + 1) * 512],
                    start=(kt == 0), stop=(kt == nk - 1),
                )

        pending = []
        for kt in range(nk):
            ps = ps_pool.tile([128, 1024], F32, tag="mm", name="ps")
            for po in (0, 64):
                nc.tensor.matmul(
                    ps[:, (po // 64) * 512:(po // 64 + 1) * 512],
                    lhsT=kT_t[ti][po:po + 64, kt * 128:(kt + 1) * 128],
                    rhs=qT_t[ti][po:po + 64, qg * 512:(qg + 1) * 512],
                    start=True, stop=True,
                )
            ex = ex_pool.tile([128, 1024], BF16, tag="ex", name="ex")
            nc.scalar.activation(out=ex, in_=ps,
                                 func=mybir.ActivationFunctionType.Exp,
                                 scale=0.125)
            if kt >= 4 * qg:  # diagonal strip: causal 0/1 mask (dup halves)
                nc.vector.tensor_mul(ex, ex, mask_t[kt - 4 * qg])
            pending.append((kt, ex))
            if len(pending) > 2:  # lag 2: AV never waits on a fresh exp
                emit_av(*pending.pop(0))

        # The remaining AV matmuls wait on the freshest exps; returning them
        # as a closure lets the caller slip an independent filler group in
        # front, so the PE chews filler instead of stalling on the ACT.
        def flush_tail():
            for item in pending:
                emit_av(*item)
            _stash(hp, qg, ti, avA, avB)
        return flush_tail

    def _stash(hp, qg, ti, avA, avB):
        # stash unnormalized outputs + denominators; release av quickly
        hA, hB = 2 * hp, 2 * hp + 1
        if qg == NQG - 1 and hp == HL // 2 - 1:
            # final pair: no attention left to hide the DRAM-bounce latency
            # behind, so normalize inline via reciprocal + PE broadcast
            stg2 = rec_pool.tile([64, 512], F32, tag="stg2", name="stg2")
            nc.vector.memset(stg2, 1.0)
            for av, po, row in ((avA, 0, 0), (avB, 64, 32)):
                nc.vector.tensor_copy(
                    outT_t[ti][po:po + 64, qg * 512:(qg + 1) * 512],
                    av[0:64, :])
                nc.vector.tensor_copy(stg2[row:row + 1, :], av[64:65, :])
            rec2 = rec_pool.tile([64, 512], F32, tag="rec2", name="rec2")
            nc.vector.reciprocal_approx_fast(out=rec2, in_=stg2)
            recb2 = rec_pool.tile([64, 512], BF16, tag="recb2", name="recb2")
            nc.vector.tensor_copy(recb2, rec2)
            bc = av_pool.tile([128, 512], F32, tag="av", name="bc")
            nc.tensor.matmul(bc, lhsT=ones2, rhs=recb2, start=True, stop=True)
            for po in (0, 64):
                sl = outT_t[ti][po:po + 64, qg * 512:(qg + 1) * 512]
                nc.vector.tensor_mul(sl, sl, bc[po:po + 64, :])
        else:
            for av, h, po in ((avA, hA, 0), (avB, hB, 64)):
                nc.vector.tensor_copy(
                    outT_t[ti][po:po + 64, qg * 512:(qg + 1) * 512],
                    av[0:64, :])
                stg = rec_pool.tile([1, 512], F32, tag="stg", name="stg",
                                    bufs=4)
                nc.vector.tensor_copy(stg, av[64:65, :])
                nc.sync.dma_start(out=sums_dram[qg, h], in_=stg)

    # ---- batched normalization (DRAM-bounce broadcast) -------------------
    def _norm_heads(qg, heads):
        h0, nh = heads[0], len(heads)
        sums = rec_pool.tile([nh, 512], F32, tag=f"sums{nh}", name="sums")
        nc.sync.dma_start(out=sums, in_=sums_dram[qg, h0:h0 + nh])
        rec = rec_pool.tile([nh, 512], F32, tag=f"rec{nh}", name="rec")
        nc.vector.reciprocal_approx_fast(out=rec, in_=sums)
        recb = rec_pool.tile([nh, 512], BF16, tag=f"recb{nh}", name="recb")
        nc.vector.tensor_copy(recb, rec)
        nc.sync.dma_start(out=rec_dram[qg, h0:h0 + nh], in_=recb)
        for h in heads:
            ti, po = h // 2, 64 * (h % 2)
            # walrus requires SBUF tensor_tensor inputs to share the start
            # partition, so land the broadcast at the same partition range
            bcs = rec_pool.tile([128, 512], BF16, tag="bcs", name="bcs")
            nc.sync.dma_start(
                out=bcs[po:po + 64, :],
                in_=rec_dram[qg, h:h + 1, :].to_broadcast([64, 512]))
            sl = outT_t[ti][po:po + 64, qg * 512:(qg + 1) * 512]
            nc.vector.tensor_mul(sl, sl, bcs[po:po + 64, :])

    def normalize(qg):
        _norm_heads(qg, list(range(HL)))

    def normalize_pair(qg, hp):
        _norm_heads(qg, [2 * hp, 2 * hp + 1])

    # ---- o-projection: y[s,:] partial ------------------------------------
    # ---- o-projection: y[s,:] partial ------------------------------------
    def oproj(st):
        ps = ps_pool.tile([128, 1024], F32, tag="mm", name="ps")
        for dt_ in range(NET):
            for hf in range(2):
                nc.tensor.matmul(
                    ps[:, hf * 512:(hf + 1) * 512],
                    lhsT=outT_t[dt_][:, st * 128:(st + 1) * 128],
                    rhs=woT_t[dt_][:, hf * 512:(hf + 1) * 512],
                    start=(dt_ == 0), stop=(dt_ == NET - 1),
                )
        ysb = y_pool.tile([128, 1024], F32, tag="ysb", name="ysb")
        # split copy+DMA per half so the writeback starts earlier
        for hf in range(2):
            nc.vector.tensor_copy(ysb[:, hf * 512:(hf + 1) * 512],
                                  ps[:, hf * 512:(hf + 1) * 512])
            nc.gpsimd.dma_start(
                out=y[st * 128:(st + 1) * 128, hf * 512:(hf + 1) * 512],
                in_=ysb[:, hf * 512:(hf + 1) * 512])

    # ---- program order ----------------------------------------------------
    # Attention is ACT(exp)-bound, so start it as soon as its first
    # dependencies exist (qg0 needs only q/k et0 cols 0:512 and v st0..3)
    # and drip the remaining PE-only projection work as filler between
    # head-pair slots, where it soaks up the PE's wait-on-exp slack.
    def qk_pair(et, scg):
        proj_qk(wqT_t, qT_t, et, scg)
        proj_qk(wkT_t, kT_t, et, scg)

    proj_v(0)
    proj_v(1)
    qk_pair(0, 0)

    fillers = {
        (0, 0): [lambda: qk_pair(1, 0), lambda: proj_v(2)],
        (0, 1): [lambda: qk_pair(2, 0), lambda: proj_v(3)],
        (0, 2): [lambda: qk_pair(3, 0), lambda: proj_v(4)],
        (0, 3): [lambda: proj_v(5)],
        (1, 0): [lambda: proj_v(6)],
        (1, 1): [lambda: proj_v(7)],
        (1, 2): [lambda: qk_pair(0, 1)],
        (1, 3): [lambda: qk_pair(1, 1)],
        (2, 0): [lambda: qk_pair(2, 1), lambda: normalize(0)],
        (2, 1): [lambda: qk_pair(3, 1), lambda: oproj(0)],
        (2, 2): [lambda: normalize(1), lambda: oproj(1)],
        (2, 3): [lambda: oproj(2), lambda: oproj(3)],
        # NOTE: a slot's fillers run AFTER its stash (flush_tail) — the
        # sums_bounce RAW dep is ordered only by sync-queue program order,
        # so normalize_pair(qg, hp) must never precede its own stash.
        (3, 0): [lambda: normalize_pair(3, 0), lambda: normalize(2),
                 lambda: oproj(4)],
        (3, 1): [lambda: normalize_pair(3, 1), lambda: oproj(5),
                 lambda: oproj(6)],
        (3, 2): [lambda: normalize_pair(3, 2), lambda: oproj(7),
                 lambda: oproj(8)],
        (3, 3): [lambda: oproj(9), lambda: oproj(10), lambda: oproj(11)],
    }
    for qg in range(NQG):
        for hp in range(HL // 2):
            flush_tail = attn(hp, qg)
            flush_tail()
            for f in fillers.get((qg, hp), []):
                f()
    for st in range(4 * (NQG - 1), 4 * NQG):
        oproj(st)


def _build():
    nc = bacc.Bacc("TRN2", target_bir_lowering=False, debug=False,
                   num_devices=NCORES)
    xT = nc.dram_tensor("xT", [D, S], BF16, kind="ExternalInput").ap()
    wqT = nc.dram_tensor("wqT", [D, E], BF16, kind="ExternalInput").ap()
    wkT = nc.dram_tensor("wkT", [D, E], BF16, kind="ExternalInput").ap()
    wvT = nc.dram_tensor("wvT", [D, E], BF16, kind="ExternalInput").ap()
    woT = nc.dram_tensor("woT", [E, D], BF16, kind="ExternalInput").ap()
    masks2 = nc.dram_tensor("masks2", [4, 128, 1024], BF16,
                            kind="ExternalInput").ap()
    y = nc.dram_tensor("y", [S, D], F32, kind="ExternalOutput").ap()
    with tile.TileContext(nc) as tc:
        _mhsa_kernel(tc, y, xT, wqT, wkT, wvT, woT, masks2)
    nc.compile()
    return nc


def get_compiled():
    global _compiled
    if _compiled is None:
        _compiled = _build()
    return _compiled


def _make_masks():
    # masks2[r][i, :] keeps key 128*r+i <= query j within the 512-wide
    # query group; duplicated in both 512-halves (head A | head B).
    m = np.zeros((4, 128, 1024), dtype=np.float32)
    col = np.arange(512)
    for r in range(4):
        half = (col[None, :] >= (128 * r + np.arange(128))[:, None])
        m[r, :, 0:512] = half
        m[r, :, 512:1024] = half
    return m.astype(bf16)


def kernel(**inputs):
    global last_results
    x = np.asarray(inputs["in_features"], dtype=np.float32)
    w_q = np.asarray(inputs["w_q"], dtype=np.float32)
    w_k = np.asarray(inputs["w_k"], dtype=np.float32)
    w_v = np.asarray(inputs["w_v"], dtype=np.float32)
    w_o = np.asarray(inputs["w_o"], dtype=np.float32)

    nc = get_compiled()
    masks2 = _make_masks()
    in_maps = []
    for c in range(NCORES):
        b, hg = divmod(c, 2)
        es = slice(hg * E, (hg + 1) * E)
        in_maps.append({
            "xT": x[b].T.astype(bf16),
            "wqT": w_q[es, :].T.astype(bf16),
            "wkT": w_k[es, :].T.astype(bf16),
            "wvT": w_v[es, :].T.astype(bf16),
            "woT": w_o[:, es].T.astype(bf16),
            "masks2": masks2,
        })
    res = run_bass_kernel_spmd(nc, in_maps, list(range(NCORES)))
    last_results = res
    y = np.zeros((B, S, D), dtype=np.float32)
    for c in range(NCORES):
        y[c // 2] += res.results[c]["y"]
    return y
